# revision 1
# baseline (speedup 1.0000x reference)
"""H2GCN forward pass on 8 Trainium2 NeuronCores (Bass/Tile SPMD kernel).

Strategy (1D row-parallel SpMM):
  - Nodes are sharded across 8 cores (1024 rows each). Each core receives the
    column-slice adjT = adj[rows, :].T (i.e. [8192, 1024]) of both adjacency
    matrices in fp16 - exactly the rhs layout the tensor engine wants, so no
    device-side transposes of the big matrices are needed.
  - The feature embed is *replicated*: every core computes the full h for all
    8192 nodes directly in node-major layout (27us of PE) instead of
    all-gathering it (~95us of exposed collective latency at kernel start).
  - conv1 computes zT = [A@h; A2@h].T feature-major, RAW (un-normalized).
  - BatchNorm is *algebraically absorbed* into the final projection:
        z_n = z*c + d  with c = gamma*rsqrt(var+eps), d = beta - mean*c
        A@z_n = (A@z)*c + rowsum(A) (x) d
    so conv2 runs on raw z, and the final projection uses per-row scaled
    weights plus three rank-1 corrections (computed from d and host-provided
    exact rowsums). This keeps every big tensor free of fixup passes.
  - Raw z is AllGathered in two feature-halves: z1's gather hides under
    conv1's second half, z2's gather hides under conv2's first m-pass (which
    only needs z1). conv2 therefore streams the adjacencies twice (DMA has
    headroom there). BN statistics use a tiny AllReduce, off critical path.
  - All matmul operands are fp16 (fp32 accumulation in PSUM): ~5e-3 relative
    error vs the fp32 reference, at 4x the fp32 matmul throughput and half the
    HBM traffic.
"""

import numpy as np

import concourse.bass as bass
import concourse.mybir as mybir
import concourse.tile as tile
from concourse import bacc
from concourse.bass_utils import run_bass_kernel_spmd
from concourse.masks import make_identity

P = 128
NCORES = 8
BN_EPS = 1e-5

F16 = mybir.dt.float16
F32 = mybir.dt.float32

FULL_CFG = dict(NT=8192, R=1024)
IN_CH = 512   # input features
H = 256       # hidden
H2 = 512      # 2*H (BN width)
O = 64        # output features
F = 7 * H     # 1792, JK concat width


def _nchunks(R):
    """Split the per-core node free-dim R into <=512 chunks (PSUM bank width)."""
    out = []
    s = 0
    while s < R:
        w = min(512, R - s)
        out.append((s, w))
        s += w
    return out


def build_program(NT, R):
    """Build the SPMD Bass program. NT = total nodes, R = rows per core."""
    KT = NT // P           # node k-tiles (contraction tiles)
    RT = R // P            # per-core node tiles (free-dim tiles / transposes)
    NCH = _nchunks(R)
    NC2 = len(NCH)
    HM = H // P            # 2  (hidden chunks)
    H2M = H2 // P          # 4
    FM = F // P            # 14
    INK = IN_CH // P       # 4

    nc = bacc.Bacc("TRN2", target_bir_lowering=False, debug=False,
                   num_devices=NCORES)

    # --- I/O -------------------------------------------------------------
    xTf = nc.dram_tensor("xTf", [IN_CH, NT], F16, kind="ExternalInput")
    xT = nc.dram_tensor("xT", [IN_CH, R], F16, kind="ExternalInput")
    adjT = nc.dram_tensor("adjT", [NT, R], F16, kind="ExternalInput")
    adjT2 = nc.dram_tensor("adjT2", [NT, R], F16, kind="ExternalInput")
    wTe = nc.dram_tensor("wTe", [IN_CH, H], F16, kind="ExternalInput")
    be = nc.dram_tensor("be", [P, HM], F32, kind="ExternalInput")
    bebc = nc.dram_tensor("bebc", [P, H], F32, kind="ExternalInput")
    wTf = nc.dram_tensor("wTf", [F, O], F16, kind="ExternalInput")
    bff = nc.dram_tensor("bff", [O, 1], F32, kind="ExternalInput")
    gam = nc.dram_tensor("gam", [P, H2M], F32, kind="ExternalInput")
    bet = nc.dram_tensor("bet", [P, H2M], F32, kind="ExternalInput")
    rsA = nc.dram_tensor("rsA", [O, R], F32, kind="ExternalInput")
    rsA2 = nc.dram_tensor("rsA2", [O, R], F32, kind="ExternalInput")
    out = nc.dram_tensor("out", [R, O], F32, kind="ExternalOutput")

    rg = [list(range(NCORES))]

    with tile.TileContext(nc) as tc:
        with (
            tc.tile_pool(name="const", bufs=1) as const,
            tc.tile_pool(name="feat", bufs=1) as feat,
            tc.tile_pool(name="tmp", bufs=2) as tmp,
            tc.tile_pool(name="stream", bufs=10) as stream,
            tc.tile_pool(name="ps", bufs=1, space="PSUM") as ps,
            tc.tile_pool(name="dram", bufs=1, space="DRAM") as dram,
        ):
            # --- constants / weights (embed-critical ones first) --------
            wTe_sb = const.tile([P, INK, H], F16, name="wTe_sb")
            nc.sync.dma_start(wTe_sb[:], wTe.ap().rearrange("(k p) m -> p k m", p=P))
            bebc_sb = const.tile([P, H], F32, name="bebc_sb")
            nc.sync.dma_start(bebc_sb[:], bebc.ap())
            xT_sb = const.tile([P, INK, R], F16, name="xT_sb")
            nc.sync.dma_start(xT_sb[:], xT.ap().rearrange("(k p) n -> p k n", p=P))
            be_sb = const.tile([P, HM], F32, name="be_sb")
            nc.sync.dma_start(be_sb[:], be.ap())
            id16 = const.tile([P, P], F16, name="id16")
            make_identity(nc, id16)
            id32 = const.tile([P, P], F32, name="id32")
            make_identity(nc, id32)

            # full x.T, loaded in node-chunk groups so the embed can start
            # before the whole 8 MB lands
            xTf_t = xTf.ap().rearrange("(k p) n -> p k n", p=P)
            xTf_sb = feat.tile([P, INK, NT], F16, name="xTf_sb", tag="kxnB")
            XGRP = max(NT // 8, P)
            for g in range(0, NT, XGRP):
                nc.sync.dma_start(xTf_sb[:, :, g:g + XGRP], xTf_t[:, :, g:g + XGRP])

            # --- phase B1: replicated full embed, node-major ------------
            # hfull[node, feat] = relu(x @ w_embed.T + b) for ALL nodes
            hfull_sb = feat.tile([P, KT, H], F16, name="hfull_sb", tag="kxnA")
            for k in range(KT):
                hps = ps.tile([P, H], F32, name=f"hps_{k}", tag=f"acc{k % 8}")
                for t in range(INK):
                    nc.tensor.matmul(
                        hps[:],
                        lhsT=xTf_sb[:, t, k * P:(k + 1) * P],
                        rhs=wTe_sb[:, t, :],
                        start=(t == 0), stop=(t == INK - 1),
                    )
                nc.vector.tensor_tensor(
                    out=hfull_sb[:, k, :], in0=hps[:], in1=bebc_sb[:],
                    op=mybir.AluOpType.add)
                nc.scalar.activation(
                    hfull_sb[:, k, :], hfull_sb[:, k, :],
                    mybir.ActivationFunctionType.Relu)

            # --- phase B2: local embed, feature-major (for the JK concat)
            hT_sb = feat.tile([P, HM, R], F16, name="hT_sb")
            for m in range(HM):
                for ci, (cs, cw) in enumerate(NCH):
                    eps_t = ps.tile([P, 512], F32, name=f"eps_{m}_{ci}",
                                    tag=f"acc{(m * NC2 + ci) % 8}")
                    for t in range(INK):
                        nc.tensor.matmul(
                            eps_t[:, :cw],
                            lhsT=wTe_sb[:, t, m * P:(m + 1) * P],
                            rhs=xT_sb[:, t, cs:cs + cw],
                            start=(t == 0), stop=(t == INK - 1),
                        )
                    nc.scalar.activation(
                        hT_sb[:, m, cs:cs + cw], eps_t[:, :cw],
                        mybir.ActivationFunctionType.Relu,
                        bias=be_sb[:, m:m + 1],
                    )

            # --- phase D: conv1, zT = [A@h; A2@h].T (raw), one adjacency
            # half at a time; z1's transposes + AllGather overlap conv1b --
            zT_sb = feat.tile([P, H2M, R], F16, name="zT_sb")
            zag_in = [None, None]
            zag_out = [None, None]
            for half, src in ((0, adjT), (1, adjT2)):
                zps = {}
                for m in range(HM):
                    for ci in range(NC2):
                        zps[(m, ci)] = ps.tile(
                            [P, 512], F32, name=f"zps_{half}_{m}_{ci}",
                            tag=f"acc{(half * 4 + m * NC2 + ci) % 8}")
                for k in range(KT):
                    at = stream.tile([P, R], F16, name=f"c1_{half}_{k}", tag="adj")
                    nc.sync.dma_start(at[:], src[k * P:(k + 1) * P, :])
                    for m in range(HM):
                        for ci, (cs, cw) in enumerate(NCH):
                            nc.tensor.matmul(
                                zps[(m, ci)][:, :cw],
                                lhsT=hfull_sb[:, k, m * P:(m + 1) * P],
                                rhs=at[:, cs:cs + cw],
                                start=(k == 0), stop=(k == KT - 1),
                            )
                for m in range(HM):
                    for ci, (cs, cw) in enumerate(NCH):
                        nc.vector.tensor_copy(
                            out=zT_sb[:, half * HM + m, cs:cs + cw],
                            in_=zps[(m, ci)][:, :cw])

                # transpose this z half to node-major and AllGather it
                z_nm = tmp.tile([P, RT, H], F16, name=f"znm_{half}", bufs=1)
                for fi in range(HM):
                    for nt in range(RT):
                        tps = ps.tile(
                            [P, P], F16, name=f"ztp_{half}_{fi}_{nt}",
                            tag=f"acc{(half * 4 + fi * RT + nt) % 4 + half * 4}")
                        nc.tensor.transpose(
                            tps[:], zT_sb[:, half * HM + fi, nt * P:(nt + 1) * P],
                            id16[:])
                        nc.any.tensor_copy(
                            out=z_nm[:, nt, fi * P:(fi + 1) * P], in_=tps[:])
                zin = dram.tile([R, H], F16, name=f"zag_in_{half}")
                nc.gpsimd.dma_start(
                    zin.rearrange("(nt p) f -> p nt f", p=P), z_nm[:])
                zout = dram.tile([NCORES, R, H], F16, name=f"zag_out_{half}")
                nc.gpsimd.collective_compute(
                    "AllGather", mybir.AluOpType.bypass, replica_groups=rg,
                    ins=[zin.opt()], outs=[zout.opt()],
                )
                zag_in[half], zag_out[half] = zin, zout

            # --- phase E: BN stats + AllReduce (off critical path) ------
            stat_sb = tmp.tile([P, 2 * H2M], F32, name="stat_sb", bufs=1)
            for f in range(H2M):
                sq = tmp.tile([P, R], F16, name="sq", tag="sq", bufs=2)
                nc.scalar.activation(
                    sq[:], zT_sb[:, f, :], mybir.ActivationFunctionType.Copy,
                    accum_out=stat_sb[:, f:f + 1])
                sq2 = tmp.tile([P, R], F16, name="sq2", tag="sq", bufs=2)
                nc.scalar.activation(
                    sq2[:], zT_sb[:, f, :], mybir.ActivationFunctionType.Square,
                    accum_out=stat_sb[:, H2M + f:H2M + f + 1])
            ar_in = dram.tile([P, 2 * H2M], F32, name="ar_in")
            nc.gpsimd.dma_start(ar_in[:], stat_sb[:])
            ar_out = dram.tile([P, 2 * H2M], F32, name="ar_out")
            nc.gpsimd.collective_compute(
                "AllReduce", mybir.AluOpType.add, replica_groups=rg,
                ins=[ar_in.opt()], outs=[ar_out.opt()],
            )
            stat_g = tmp.tile([P, 2 * H2M], F32, name="stat_g", bufs=1)
            nc.gpsimd.dma_start(stat_g[:], ar_out[:])

            # BN coefficients c, d (feature-major [128, 4], fp32)
            gam_sb = const.tile([P, H2M], F32, name="gam_sb")
            nc.sync.dma_start(gam_sb[:], gam.ap())
            bet_sb = const.tile([P, H2M], F32, name="bet_sb")
            nc.sync.dma_start(bet_sb[:], bet.ap())
            cmean = tmp.tile([P, H2M], F32, name="cmean", bufs=1)
            nc.scalar.mul(cmean[:], stat_g[:, 0:H2M], 1.0 / NT)
            cvar = tmp.tile([P, H2M], F32, name="cvar", bufs=1)
            nc.scalar.mul(cvar[:], stat_g[:, H2M:2 * H2M], 1.0 / NT)
            msq = tmp.tile([P, H2M], F32, name="msq", bufs=1)
            nc.vector.tensor_mul(out=msq[:], in0=cmean[:], in1=cmean[:])
            nc.vector.tensor_tensor(
                out=cvar[:], in0=cvar[:], in1=msq[:],
                op=mybir.AluOpType.subtract)
            eps_sb = tmp.tile([P, 1], F32, name="eps_sb", bufs=1)
            nc.vector.memset(eps_sb[:], BN_EPS)
            cstd = tmp.tile([P, H2M], F32, name="cstd", bufs=1)
            nc.scalar.activation(
                cstd[:], cvar[:], mybir.ActivationFunctionType.Sqrt,
                bias=eps_sb[:])
            crstd = tmp.tile([P, H2M], F32, name="crstd", bufs=1)
            nc.vector.reciprocal(crstd[:], cstd[:])
            c_t = tmp.tile([P, H2M], F32, name="c_t", bufs=1)
            nc.vector.tensor_mul(out=c_t[:], in0=crstd[:], in1=gam_sb[:])
            d_t = tmp.tile([P, H2M], F32, name="d_t", bufs=1)
            nc.vector.tensor_mul(out=d_t[:], in0=cmean[:], in1=c_t[:])
            nc.vector.tensor_tensor(
                out=d_t[:], in0=bet_sb[:], in1=d_t[:],
                op=mybir.AluOpType.subtract)
            d16 = tmp.tile([P, H2M], F16, name="d16", bufs=1)
            nc.vector.tensor_copy(out=d16[:], in_=d_t[:])

            # --- phase F: conv2 on raw z, two m-half passes -------------
            # pass 0 (z features 0:256, from z1) only needs zag_out[0], so
            # it overlaps z2's AllGather; each pass streams both adjacencies
            u_sb = feat.tile([P, 2 * H2M, R], F16, name="u_sb")
            for half in (0, 1):
                zf_sb = feat.tile([P, KT, H], F16, name=f"zf_{half}",
                                  tag=("kxnB" if half == 0 else "kxnA"))
                nc.gpsimd.dma_start(
                    zf_sb[:],
                    zag_out[half].rearrange("r (nt p) f -> p (r nt) f", p=P))
                ups = {}
                for a in (0, 1):
                    for m in range(HM):
                        for ci in range(NC2):
                            ups[(a, m, ci)] = ps.tile(
                                [P, 512], F32, name=f"ups_{half}_{a}_{m}_{ci}",
                                tag=f"acc{(a * 4 + m * NC2 + ci) % 8}")
                for k in range(KT):
                    at = stream.tile([P, R], F16, name=f"c2a_{half}_{k}", tag="adj")
                    nc.sync.dma_start(at[:], adjT[k * P:(k + 1) * P, :])
                    at2 = stream.tile([P, R], F16, name=f"c2b_{half}_{k}", tag="adj")
                    nc.sync.dma_start(at2[:], adjT2[k * P:(k + 1) * P, :])
                    for m in range(HM):
                        for ci, (cs, cw) in enumerate(NCH):
                            nc.tensor.matmul(
                                ups[(0, m, ci)][:, :cw],
                                lhsT=zf_sb[:, k, m * P:(m + 1) * P],
                                rhs=at[:, cs:cs + cw],
                                start=(k == 0), stop=(k == KT - 1),
                            )
                            nc.tensor.matmul(
                                ups[(1, m, ci)][:, :cw],
                                lhsT=zf_sb[:, k, m * P:(m + 1) * P],
                                rhs=at2[:, cs:cs + cw],
                                start=(k == 0), stop=(k == KT - 1),
                            )
                # u feature layout: chunks 0..3 = U1 (A@z), 4..7 = U2 (A2@z);
                # this pass produces z-feature chunks {half*2, half*2+1} of each
                for a in (0, 1):
                    for m in range(HM):
                        for ci, (cs, cw) in enumerate(NCH):
                            nc.vector.tensor_copy(
                                out=u_sb[:, a * H2M + half * HM + m, cs:cs + cw],
                                in_=ups[(a, m, ci)][:, :cw])

            # --- phase G: final projection with absorbed BN -------------
            wTf_sb = const.tile([P, FM, O], F16, name="wTf_sb")
            nc.sync.dma_start(wTf_sb[:], wTf.ap().rearrange("(k p) m -> p k m", p=P))
            bff_sb = const.tile([O, 1], F32, name="bff_sb")
            nc.sync.dma_start(bff_sb[:], bff.ap())
            rsA_sb = const.tile([O, R], F32, name="rsA_sb")
            nc.sync.dma_start(rsA_sb[:], rsA.ap())
            rsA2_sb = const.tile([O, R], F32, name="rsA2_sb")
            nc.sync.dma_start(rsA2_sb[:], rsA2.ap())

            # s_j = W_block_j @ d  (blocks: z_n, U1, U2), from UNSCALED wTf
            s_cols = tmp.tile([O, 3], F32, name="s_cols", bufs=1)
            for j, base in enumerate((HM, HM + H2M, HM + 2 * H2M)):
                sps = ps.tile([O, 1], F32, name=f"sps_{j}", tag=f"acc{j}")
                for t in range(H2M):
                    nc.tensor.matmul(
                        sps[:], lhsT=wTf_sb[:, base + t, :],
                        rhs=d16[:, t:t + 1],
                        start=(t == 0), stop=(t == H2M - 1))
                nc.vector.tensor_copy(out=s_cols[:, j:j + 1], in_=sps[:])
            s0b = tmp.tile([O, 1], F32, name="s0b", bufs=1)
            nc.vector.tensor_add(out=s0b[:], in0=s_cols[:, 0:1], in1=bff_sb[:])

            # scale wTf rows (z_n, U1, U2 blocks) by c, in place
            for t in range(HM, FM):
                ch = (t - HM) % H2M
                nc.vector.tensor_scalar_mul(
                    wTf_sb[:, t, :], wTf_sb[:, t, :], c_t[:, ch:ch + 1])

            # outT[64, R] = wTf'.T @ jkT + (s0+bf) + s1 (x) rsA + s2 (x) rsA2
            def jk_rhs(t):
                if t < HM:
                    return hT_sb[:, t, :]
                if t < HM + H2M:
                    return zT_sb[:, t - HM, :]
                return u_sb[:, t - HM - H2M, :]

            outsb = tmp.tile([O, R], F32, name="outsb", bufs=1)
            for ci, (cs, cw) in enumerate(NCH):
                ops = ps.tile([O, 512], F32, name=f"ops_{ci}", tag=f"acc{4 + ci}")
                for t in range(FM):
                    nc.tensor.matmul(
                        ops[:, :cw], lhsT=wTf_sb[:, t, :],
                        rhs=jk_rhs(t)[:, cs:cs + cw],
                        start=(t == 0), stop=(t == FM - 1))
                nc.vector.tensor_scalar_add(
                    outsb[:, cs:cs + cw], ops[:, :cw], s0b[:])
            rk1 = tmp.tile([O, R], F32, name="rk1", bufs=1)
            nc.vector.tensor_scalar_mul(rk1[:], rsA_sb[:], s_cols[:, 1:2])
            nc.vector.tensor_add(out=outsb[:], in0=outsb[:], in1=rk1[:])
            rk2 = tmp.tile([O, R], F32, name="rk2", bufs=1)
            nc.vector.tensor_scalar_mul(rk2[:], rsA2_sb[:], s_cols[:, 2:3])
            nc.vector.tensor_add(out=outsb[:], in0=outsb[:], in1=rk2[:])

            # transpose [O, R] -> node-major [R, O] and write out
            o_nm = tmp.tile([P, RT, O], F32, name="o_nm", bufs=1)
            for nt in range(RT):
                tps32 = ps.tile([P, O], F32, name=f"otp_{nt}",
                                tag=f"acc{nt % 8}")
                nc.tensor.transpose(
                    tps32[:], outsb[:, nt * P:(nt + 1) * P], id32[:O, :O])
                nc.any.tensor_copy(out=o_nm[:, nt, :], in_=tps32[:])
            nc.sync.dma_start(
                out.ap().rearrange("(nt p) o -> p nt o", p=P), o_nm[:])

    nc.compile()
    return nc


_PROGRAM_CACHE = {}


def _get_program(NT, R):
    key = (NT, R)
    if key not in _PROGRAM_CACHE:
        _PROGRAM_CACHE[key] = build_program(NT, R)
    return _PROGRAM_CACHE[key]


def make_in_maps(inputs, NT, R):
    """Shard full inputs into per-core input maps (host-side, numpy)."""
    x = np.asarray(inputs["x"], np.float32)
    adj = np.asarray(inputs["adj_t"], np.float32)
    adj2 = np.asarray(inputs["adj_t2"], np.float32)
    we = np.asarray(inputs["w_embed"], np.float32)
    be = np.asarray(inputs["b_embed"], np.float32)
    gam = np.asarray(inputs["bn_gamma"], np.float32)
    bet = np.asarray(inputs["bn_beta"], np.float32)
    wf = np.asarray(inputs["w_fin"], np.float32)
    bf = np.asarray(inputs["b_fin"], np.float32)

    H2M = H2 // P
    xTf_h = np.ascontiguousarray(x.T).astype(np.float16)
    wTe_h = np.ascontiguousarray(we.T).astype(np.float16)
    be_h = np.ascontiguousarray(be.reshape(H // P, P).T).astype(np.float32)
    bebc_h = np.ascontiguousarray(
        np.broadcast_to(be[None, :], (P, H))).astype(np.float32)
    wTf_h = np.ascontiguousarray(wf.T).astype(np.float16)
    bff_h = np.ascontiguousarray(bf[:, None]).astype(np.float32)
    gam_h = np.ascontiguousarray(gam.reshape(H2M, P).T).astype(np.float32)
    bet_h = np.ascontiguousarray(bet.reshape(H2M, P).T).astype(np.float32)

    in_maps = []
    for r in range(NCORES):
        rows = slice(r * R, (r + 1) * R)
        rsA_h = np.ascontiguousarray(
            np.broadcast_to(adj[rows].sum(1)[None, :], (O, R))).astype(np.float32)
        rsA2_h = np.ascontiguousarray(
            np.broadcast_to(adj2[rows].sum(1)[None, :], (O, R))).astype(np.float32)
        in_maps.append({
            "xTf": xTf_h,
            "xT": np.ascontiguousarray(x[rows].T).astype(np.float16),
            "adjT": np.ascontiguousarray(adj[rows].T).astype(np.float16),
            "adjT2": np.ascontiguousarray(adj2[rows].T).astype(np.float16),
            "wTe": wTe_h, "be": be_h, "bebc": bebc_h, "wTf": wTf_h,
            "bff": bff_h, "gam": gam_h, "bet": bet_h,
            "rsA": rsA_h, "rsA2": rsA2_h,
        })
    return in_maps


def kernel(**inputs):
    NT, R = FULL_CFG["NT"], FULL_CFG["R"]
    nc = _get_program(NT, R)
    in_maps = make_in_maps(inputs, NT, R)
    res = run_bass_kernel_spmd(nc, in_maps, core_ids=list(range(NCORES)))
    out = np.concatenate(
        [res.results[r]["out"] for r in range(NCORES)], axis=0)
    return out.astype(np.float32)



# revision 5
# speedup vs baseline: 1.5420x; 1.5420x over previous
"""H2GCN forward pass on 8 Trainium2 NeuronCores (Bass/Tile SPMD kernel).

Strategy (1D row-parallel SpMM, restructured):
  - Nodes sharded across 8 cores (1024 rows each). Both adjacencies are
    decomposed exactly as A_norm = diag(dis) @ A01 @ diag(dis) with A01 the
    0/1 edge mask and dis = d^-1/2.  A01 is streamed as fp8 (0 and 1 are
    exact in e4m3) at half the fp16 bytes; the dis scalings fold into the
    fp16 lhsT operands and the PSUM->SBUF copy-outs, so the SpMMs carry NO
    quantization error beyond fp16.
  - conv1: z.T = [A@h; A2@h].T computed feature-major with lhsT = dis_a*h
    (replicated full embed, fp16) x fp8 adjT tiles (mixed-dtype matmul).
  - BatchNorm is absorbed: z_n = z*c + d.  A tiny AllReduce of the z
    stats gives c,d.
  - conv2 is ASSOCIATED into the final projection: the U1/U2 JK blocks only
    feed w_fin, and (A @ z_n) @ Wu^T = A @ (z_n @ Wu^T) = A @ Y with Y only
    64 wide.  Each core computes Y for its rows (z_n @ (Wu c)^T + 1*(Wu d)),
    scales by dis_a, AllGathers the tiny [8192, 2x64] fp16 Y, and runs the
    second SpMM 64-wide -- 4x less PE work than the naive 512-wide conv2,
    one adjacency pass instead of two, and a 16x smaller collective.
  - Final projection: out.T = Wh@h.T + (Wz c)@z.T + (Wz@d + bf) + dis*(A01@Ys).
"""

import numpy as np
import ml_dtypes

import concourse.bass as bass
import concourse.mybir as mybir
import concourse.tile as tile
from concourse import bacc
from concourse.bass_utils import run_bass_kernel_spmd
from concourse.masks import make_identity

P = 128
NCORES = 8
BN_EPS = 1e-5

F8 = mybir.dt.float8e4
F16 = mybir.dt.float16
F32 = mybir.dt.float32
NPF8 = ml_dtypes.float8_e4m3

FULL_CFG = dict(NT=8192, R=1024)
IN_CH = 512   # input features
H = 256       # hidden
H2 = 512      # 2*H (BN width)
O = 64        # output features
FM = 14       # 7*H/128 JK weight chunks: [h:0-1, z_n:2-5, U1:6-9, U2:10-13]


def build_program(NT, R):
    KT = NT // P           # 64 global node k-tiles
    KT2 = KT // 2          # 32 paired tiles (fp8 stream layout)
    RT = R // P            # 8 local node tiles
    NCH = [(0, 512), (512, 512)]
    HM = H // P            # 2
    H2M = H2 // P          # 4
    INK = IN_CH // P       # 4

    nc = bacc.Bacc("TRN2", target_bir_lowering=False, debug=False,
                   num_devices=NCORES)

    # --- I/O -------------------------------------------------------------
    xTf = nc.dram_tensor("xTf", [IN_CH, NT], F16, kind="ExternalInput")
    xT = nc.dram_tensor("xT", [IN_CH, R], F16, kind="ExternalInput")
    adjp1 = nc.dram_tensor("adjp1", [KT2 * P, 2 * R], F8, kind="ExternalInput")
    adjp2 = nc.dram_tensor("adjp2", [KT2 * P, 2 * R], F8, kind="ExternalInput")
    wTe = nc.dram_tensor("wTe", [IN_CH, H], F16, kind="ExternalInput")
    be = nc.dram_tensor("be", [P, HM], F32, kind="ExternalInput")
    bebc = nc.dram_tensor("bebc", [P, H], F32, kind="ExternalInput")
    wTf = nc.dram_tensor("wTf", [7 * H, O], F16, kind="ExternalInput")
    bff = nc.dram_tensor("bff", [O, 1], F32, kind="ExternalInput")
    gam = nc.dram_tensor("gam", [P, H2M], F32, kind="ExternalInput")
    bet = nc.dram_tensor("bet", [P, H2M], F32, kind="ExternalInput")
    disP1 = nc.dram_tensor("disP1", [P, KT], F32, kind="ExternalInput")
    rdisP = nc.dram_tensor("rdisP", [P, KT], F32, kind="ExternalInput")
    cu1 = nc.dram_tensor("cu1", [P, R], F32, kind="ExternalInput")
    cu2 = nc.dram_tensor("cu2", [P, R], F32, kind="ExternalInput")
    disNM = nc.dram_tensor("disNM", [P, 2 * RT], F32, kind="ExternalInput")
    disRO1 = nc.dram_tensor("disRO1", [O, R], F32, kind="ExternalInput")
    disRO2 = nc.dram_tensor("disRO2", [O, R], F32, kind="ExternalInput")
    out = nc.dram_tensor("out", [R, O], F32, kind="ExternalOutput")

    rg = [list(range(NCORES))]

    with tile.TileContext(nc) as tc:
        with (
            tc.tile_pool(name="const", bufs=1) as const,
            tc.tile_pool(name="feat", bufs=1) as feat,
            tc.tile_pool(name="tmp", bufs=2) as tmp,
            tc.tile_pool(name="stream", bufs=12) as stream,
            tc.tile_pool(name="ps", bufs=1, space="PSUM") as ps,
            tc.tile_pool(name="dram", bufs=1, space="DRAM") as dram,
        ):
            # --- constants / weights --------------------------------------
            wTe_sb = const.tile([P, INK, H], F16, name="wTe_sb")
            nc.sync.dma_start(wTe_sb[:], wTe.ap().rearrange("(k p) m -> p k m", p=P))
            bebc_sb = const.tile([P, H], F32, name="bebc_sb")
            nc.sync.dma_start(bebc_sb[:], bebc.ap())
            be_sb = const.tile([P, HM], F32, name="be_sb")
            nc.sync.dma_start(be_sb[:], be.ap())
            disP1_sb = const.tile([P, KT], F32, name="disP1_sb")
            nc.sync.dma_start(disP1_sb[:], disP1.ap())
            rdisP_sb = const.tile([P, KT], F32, name="rdisP_sb")
            nc.sync.dma_start(rdisP_sb[:], rdisP.ap())
            cu_sb = [const.tile([P, R], F32, name=f"cu{a}_sb") for a in (0, 1)]
            nc.sync.dma_start(cu_sb[0][:], cu1.ap())
            nc.sync.dma_start(cu_sb[1][:], cu2.ap())
            xT_sb = const.tile([P, INK, R], F16, name="xT_sb")
            nc.sync.dma_start(xT_sb[:], xT.ap().rearrange("(k p) n -> p k n", p=P))
            disNM_sb = const.tile([P, 2 * RT], F32, name="disNM_sb")
            nc.sync.dma_start(disNM_sb[:], disNM.ap())
            gam_sb = const.tile([P, H2M], F32, name="gam_sb")
            nc.sync.dma_start(gam_sb[:], gam.ap())
            bet_sb = const.tile([P, H2M], F32, name="bet_sb")
            nc.sync.dma_start(bet_sb[:], bet.ap())
            wTf_sb = const.tile([P, FM, O], F16, name="wTf_sb")
            nc.sync.dma_start(wTf_sb[:], wTf.ap().rearrange("(k p) m -> p k m", p=P))
            bff_sb = const.tile([O, 1], F32, name="bff_sb")
            nc.sync.dma_start(bff_sb[:], bff.ap())
            disRO_sb = [const.tile([O, R], F32, name=f"disRO{a}_sb") for a in (0, 1)]
            nc.sync.dma_start(disRO_sb[0][:], disRO1.ap())
            nc.sync.dma_start(disRO_sb[1][:], disRO2.ap())
            id32 = const.tile([P, P], F32, name="id32")
            make_identity(nc, id32)
            ones1 = const.tile([1, P], F16, name="ones1")
            nc.vector.memset(ones1[:], 1.0)

            # full x.T in node-chunk groups so the embed starts early
            xTf_t = xTf.ap().rearrange("(k p) n -> p k n", p=P)
            xTf_sb = feat.tile([P, INK, NT], F16, name="xTf_sb", tag="big64")
            XGRP = NT // 8
            for g in range(0, NT, XGRP):
                nc.sync.dma_start(xTf_sb[:, :, g:g + XGRP], xTf_t[:, :, g:g + XGRP])

            # --- B1: replicated full embed -> hs1 = dis1*relu(x@We.T+b) ---
            hs1 = feat.tile([P, KT, H], F16, name="hs1")
            for k in range(KT):
                hps = ps.tile([P, H], F32, name=f"hps_{k}", tag=f"acc{k % 4}")
                for t in range(INK):
                    nc.tensor.matmul(
                        hps[:],
                        lhsT=xTf_sb[:, t, k * P:(k + 1) * P],
                        rhs=wTe_sb[:, t, :],
                        start=(t == 0), stop=(t == INK - 1),
                    )
                ht = tmp.tile([P, H], F16, name=f"ht_{k}", tag="htmp", bufs=3)
                nc.vector.tensor_tensor(
                    out=ht[:], in0=hps[:], in1=bebc_sb[:],
                    op=mybir.AluOpType.add)
                nc.scalar.activation(
                    ht[:], ht[:], mybir.ActivationFunctionType.Relu)
                nc.vector.tensor_scalar_mul(
                    hs1[:, k, :], ht[:], disP1_sb[:, k:k + 1])

            # --- B2: local embed, feature-major (JK h block) --------------
            hT_sb = feat.tile([P, HM, R], F16, name="hT_sb")
            for m in range(HM):
                for ci, (cs, cw) in enumerate(NCH):
                    eps_t = ps.tile([P, 512], F32, name=f"eps_{m}_{ci}",
                                    tag=f"acc{4 + (m * 2 + ci) % 4}")
                    for t in range(INK):
                        nc.tensor.matmul(
                            eps_t[:, :cw],
                            lhsT=wTe_sb[:, t, m * P:(m + 1) * P],
                            rhs=xT_sb[:, t, cs:cs + cw],
                            start=(t == 0), stop=(t == INK - 1),
                        )
                    nc.scalar.activation(
                        hT_sb[:, m, cs:cs + cw], eps_t[:, :cw],
                        mybir.ActivationFunctionType.Relu,
                        bias=be_sb[:, m:m + 1],
                    )

            # hs2 = (dis2/dis1) * hs1, built into xTf's buffer (freed now)
            hs2 = feat.tile([P, KT, H], F16, name="hs2", tag="big64")
            for k in range(KT):
                nc.vector.tensor_scalar_mul(
                    hs2[:, k, :], hs1[:, k, :], rdisP_sb[:, k:k + 1])

            # --- conv1: zT = [A@h; A2@h].T, mixed fp16 x fp8 --------------
            zT_sb = feat.tile([P, H2M, R], F16, name="zT_sb")
            stat_sb = tmp.tile([P, 2 * H2M], F32, name="stat_sb", bufs=1)
            for half, (src, hs_a) in enumerate(((adjp1, hs1), (adjp2, hs2))):
                zps = {}
                for m in range(HM):
                    for ci in range(2):
                        zps[(m, ci)] = ps.tile(
                            [P, 512], F32, name=f"zps_{half}_{m}_{ci}",
                            tag=f"acc{half * 4 + m * 2 + ci}")
                for k2 in range(KT2):
                    at = stream.tile([P, 2, R], F8, name=f"c1_{half}_{k2}",
                                     tag="adj")
                    nc.sync.dma_start(at[:], src[k2 * P:(k2 + 1) * P, :])
                    for c in range(2):
                        gk = 2 * k2 + c
                        for m in range(HM):
                            for ci, (cs, cw) in enumerate(NCH):
                                nc.tensor.matmul(
                                    zps[(m, ci)][:, :cw],
                                    lhsT=hs_a[:, gk, m * P:(m + 1) * P],
                                    rhs=at[:, c, cs:cs + cw],
                                    start=(gk == 0), stop=(gk == KT - 1),
                                )
                for m in range(HM):
                    for ci, (cs, cw) in enumerate(NCH):
                        nc.vector.tensor_tensor(
                            out=zT_sb[:, half * HM + m, cs:cs + cw],
                            in0=zps[(m, ci)][:, :cw],
                            in1=cu_sb[half][:, cs:cs + cw],
                            op=mybir.AluOpType.mult)
                # BN statistics for this half (sum, sum of squares)
                for m in range(HM):
                    f = half * HM + m
                    sq = tmp.tile([P, R], F16, name=f"sq_{f}", tag="sq", bufs=2)
                    nc.scalar.activation(
                        sq[:], zT_sb[:, f, :], mybir.ActivationFunctionType.Copy,
                        accum_out=stat_sb[:, f:f + 1])
                    sq2 = tmp.tile([P, R], F16, name=f"sq2_{f}", tag="sq", bufs=2)
                    nc.scalar.activation(
                        sq2[:], zT_sb[:, f, :],
                        mybir.ActivationFunctionType.Square,
                        accum_out=stat_sb[:, H2M + f:H2M + f + 1])

            # --- BN coefficient AllReduce --------------------------------
            ar_in = dram.tile([P, 2 * H2M], F32, name="ar_in")
            nc.gpsimd.dma_start(ar_in[:], stat_sb[:])
            ar_out = dram.tile([P, 2 * H2M], F32, name="ar_out")
            nc.gpsimd.collective_compute(
                "AllReduce", mybir.AluOpType.add, replica_groups=rg,
                ins=[ar_in.opt()], outs=[ar_out.opt()],
            )
            stat_g = tmp.tile([P, 2 * H2M], F32, name="stat_g", bufs=1)
            nc.gpsimd.dma_start(stat_g[:], ar_out[:])

            cmean = tmp.tile([P, H2M], F32, name="cmean", bufs=1)
            nc.scalar.mul(cmean[:], stat_g[:, 0:H2M], 1.0 / NT)
            cvar = tmp.tile([P, H2M], F32, name="cvar", bufs=1)
            nc.scalar.mul(cvar[:], stat_g[:, H2M:2 * H2M], 1.0 / NT)
            msq = tmp.tile([P, H2M], F32, name="msq", bufs=1)
            nc.vector.tensor_mul(out=msq[:], in0=cmean[:], in1=cmean[:])
            nc.vector.tensor_tensor(
                out=cvar[:], in0=cvar[:], in1=msq[:],
                op=mybir.AluOpType.subtract)
            eps_sb = tmp.tile([P, 1], F32, name="eps_sb", bufs=1)
            nc.vector.memset(eps_sb[:], BN_EPS)
            cstd = tmp.tile([P, H2M], F32, name="cstd", bufs=1)
            nc.scalar.activation(
                cstd[:], cvar[:], mybir.ActivationFunctionType.Sqrt,
                bias=eps_sb[:])
            crstd = tmp.tile([P, H2M], F32, name="crstd", bufs=1)
            nc.vector.reciprocal(crstd[:], cstd[:])
            c_t = tmp.tile([P, H2M], F32, name="c_t", bufs=1)
            nc.vector.tensor_mul(out=c_t[:], in0=crstd[:], in1=gam_sb[:])
            d_t = tmp.tile([P, H2M], F32, name="d_t", bufs=1)
            nc.vector.tensor_mul(out=d_t[:], in0=cmean[:], in1=c_t[:])
            nc.vector.tensor_tensor(
                out=d_t[:], in0=bet_sb[:], in1=d_t[:],
                op=mybir.AluOpType.subtract)
            d16 = tmp.tile([P, H2M], F16, name="d16", bufs=1)
            nc.vector.tensor_copy(out=d16[:], in_=d_t[:])

            # --- s0 = Wz@d + bf ; yda_a = Wu_a@d (UNSCALED wTf rows) ------
            sps = ps.tile([O, 1], F32, name="sps", tag="acc0")
            for t in range(H2M):
                nc.tensor.matmul(
                    sps[:], lhsT=wTf_sb[:, 2 + t, :], rhs=d16[:, t:t + 1],
                    start=(t == 0), stop=(t == H2M - 1))
            s0b = tmp.tile([O, 1], F32, name="s0b", bufs=1)
            nc.vector.tensor_add(out=s0b[:], in0=sps[:], in1=bff_sb[:])

            ydaBC = []
            for a in (0, 1):
                yps = ps.tile([1, O], F32, name=f"ydps_{a}", tag="acc1")
                for t in range(H2M):
                    nc.tensor.matmul(
                        yps[:], lhsT=d16[:, t:t + 1],
                        rhs=wTf_sb[:, 6 + 4 * a + t, :],
                        start=(t == 0), stop=(t == H2M - 1))
                yda16 = tmp.tile([1, O], F16, name=f"yda16_{a}", bufs=1)
                nc.vector.tensor_copy(out=yda16[:], in_=yps[:])
                ybps = ps.tile([P, O], F32, name=f"ybps_{a}", tag="acc2")
                nc.tensor.matmul(ybps[:], lhsT=ones1[:], rhs=yda16[:],
                                 start=True, stop=True)
                yb = tmp.tile([P, O], F32, name=f"ydaBC_{a}", bufs=1)
                nc.vector.tensor_copy(out=yb[:], in_=ybps[:])
                ydaBC.append(yb)

            # scale z_n/U1/U2 weight rows by c, in place
            for t in range(2, FM):
                ch = (t - 2) % H2M
                nc.vector.tensor_scalar_mul(
                    wTf_sb[:, t, :], wTf_sb[:, t, :], c_t[:, ch:ch + 1])

            # --- Ys_a = dis_a * (z @ (Wu_a c)^T + 1x(Wu_a@d)), node-major -
            ys_nm = tmp.tile([P, 2, RT, O], F16, name="ys_nm", bufs=1)
            for a in (0, 1):
                for nt in range(RT):
                    yp = ps.tile([P, O], F32, name=f"yp_{a}_{nt}",
                                 tag=f"acc{3 + (a * RT + nt) % 4}")
                    for t in range(H2M):
                        nc.tensor.matmul(
                            yp[:],
                            lhsT=zT_sb[:, t, nt * P:(nt + 1) * P],
                            rhs=wTf_sb[:, 6 + 4 * a + t, :],
                            start=(t == 0), stop=(t == H2M - 1))
                    yt = tmp.tile([P, O], F32, name=f"yt_{a}_{nt}",
                                  tag="ytmp", bufs=3)
                    nc.vector.tensor_tensor(
                        out=yt[:], in0=yp[:], in1=ydaBC[a][:],
                        op=mybir.AluOpType.add)
                    nc.vector.tensor_scalar_mul(
                        ys_nm[:, a, nt, :], yt[:],
                        disNM_sb[:, a * RT + nt:a * RT + nt + 1])

            ysin = dram.tile([2, R, O], F16, name="ysin")
            nc.gpsimd.dma_start(
                ysin.rearrange("a (nt p) f -> p (a nt) f", p=P), ys_nm[:])
            ysag = dram.tile([NCORES, 2, R, O], F16, name="ysag",
                             addr_space="Shared")
            nc.gpsimd.collective_compute(
                "AllGather", mybir.AluOpType.bypass, replica_groups=rg,
                ins=[ysin.opt()], outs=[ysag.opt()],
            )
            ysf = []
            for a in (0, 1):
                yf = feat.tile([P, NCORES, RT, O], F16, name=f"ysf_{a}")
                for r in range(NCORES):
                    nc.gpsimd.dma_start(
                        yf[:, r, :, :],
                        ysag[r, a].rearrange("(nt p) f -> p nt f", p=P))
                ysf.append(yf)

            # --- conv2: ups_a = A01_a @ Ys_a  (64-wide, one pass) ---------
            ups = {}
            for a in (0, 1):
                for ci in range(2):
                    ups[(a, ci)] = ps.tile(
                        [O, 512], F32, name=f"ups_{a}_{ci}",
                        tag=f"acc{4 + a * 2 + ci}")
            for k2 in range(KT2):
                at1 = stream.tile([P, 2, R], F8, name=f"c2a_{k2}", tag="adj")
                nc.sync.dma_start(at1[:], adjp1[k2 * P:(k2 + 1) * P, :])
                at2 = stream.tile([P, 2, R], F8, name=f"c2b_{k2}", tag="adj")
                nc.sync.dma_start(at2[:], adjp2[k2 * P:(k2 + 1) * P, :])
                for c in range(2):
                    gk = 2 * k2 + c
                    for a, at in ((0, at1), (1, at2)):
                        for ci, (cs, cw) in enumerate(NCH):
                            nc.tensor.matmul(
                                ups[(a, ci)][:, :cw],
                                lhsT=ysf[a][:, gk // RT, gk % RT, :],
                                rhs=at[:, c, cs:cs + cw],
                                start=(gk == 0), stop=(gk == KT - 1),
                            )

            # --- final projection ----------------------------------------
            outsb = tmp.tile([O, R], F32, name="outsb", bufs=1)
            for ci, (cs, cw) in enumerate(NCH):
                ops = ps.tile([O, 512], F32, name=f"ops_{ci}", tag=f"acc{ci}")
                for t in range(2):
                    nc.tensor.matmul(
                        ops[:, :cw], lhsT=wTf_sb[:, t, :],
                        rhs=hT_sb[:, t, cs:cs + cw],
                        start=(t == 0), stop=False)
                for t in range(H2M):
                    nc.tensor.matmul(
                        ops[:, :cw], lhsT=wTf_sb[:, 2 + t, :],
                        rhs=zT_sb[:, t, cs:cs + cw],
                        start=False, stop=(t == H2M - 1))
                nc.vector.tensor_scalar_add(
                    outsb[:, cs:cs + cw], ops[:, :cw], s0b[:])
            for a in (0, 1):
                va = tmp.tile([O, R], F32, name=f"va_{a}", bufs=1)
                for ci, (cs, cw) in enumerate(NCH):
                    nc.vector.tensor_tensor(
                        out=va[:, cs:cs + cw], in0=ups[(a, ci)][:, :cw],
                        in1=disRO_sb[a][:, cs:cs + cw],
                        op=mybir.AluOpType.mult)
                nc.vector.tensor_add(out=outsb[:], in0=outsb[:], in1=va[:])

            # transpose [O, R] -> node-major [R, O] and write out
            o_nm = tmp.tile([P, RT, O], F32, name="o_nm", bufs=1)
            for nt in range(RT):
                tps32 = ps.tile([P, O], F32, name=f"otp_{nt}",
                                tag=f"acc{2 + nt % 2}")
                nc.tensor.transpose(
                    tps32[:], outsb[:, nt * P:(nt + 1) * P], id32[:O, :O])
                nc.any.tensor_copy(out=o_nm[:, nt, :], in_=tps32[:])
            nc.sync.dma_start(
                out.ap().rearrange("(nt p) o -> p nt o", p=P), o_nm[:])

    nc.compile()
    return nc


_PROGRAM_CACHE = {}


def _get_program(NT, R):
    key = (NT, R)
    if key not in _PROGRAM_CACHE:
        _PROGRAM_CACHE[key] = build_program(NT, R)
    return _PROGRAM_CACHE[key]


def make_in_maps(inputs, NT, R):
    """Shard full inputs into per-core input maps (host-side, numpy)."""
    KT = NT // P
    KT2 = KT // 2
    RT = R // P
    HM = H // P
    H2M = H2 // P

    x = np.asarray(inputs["x"], np.float32)
    adj = np.asarray(inputs["adj_t"], np.float32)
    adj2 = np.asarray(inputs["adj_t2"], np.float32)
    we = np.asarray(inputs["w_embed"], np.float32)
    be_v = np.asarray(inputs["b_embed"], np.float32)
    gam_v = np.asarray(inputs["bn_gamma"], np.float32)
    bet_v = np.asarray(inputs["bn_beta"], np.float32)
    wf = np.asarray(inputs["w_fin"], np.float32)
    bf = np.asarray(inputs["b_fin"], np.float32)

    # exact 0/1 decomposition of the gcn-normalized adjacencies
    A01 = [(adj != 0), (adj2 != 0)]
    dis = []
    for A in A01:
        d = A.sum(1, dtype=np.float64).astype(np.float32)
        dis.append(np.where(d > 0, 1.0 / np.sqrt(np.maximum(d, 1e-12)), 0.0)
                   .astype(np.float32))
    rdis = np.where(dis[0] > 0, dis[1] / np.maximum(dis[0], 1e-30), 0.0
                    ).astype(np.float32)

    xTf_h = np.ascontiguousarray(x.T).astype(np.float16)
    wTe_h = np.ascontiguousarray(we.T).astype(np.float16)
    be_h = np.ascontiguousarray(be_v.reshape(HM, P).T).astype(np.float32)
    bebc_h = np.ascontiguousarray(
        np.broadcast_to(be_v[None, :], (P, H))).astype(np.float32)
    wTf_h = np.ascontiguousarray(wf.T).astype(np.float16)
    bff_h = np.ascontiguousarray(bf[:, None]).astype(np.float32)
    gam_h = np.ascontiguousarray(gam_v.reshape(H2M, P).T).astype(np.float32)
    bet_h = np.ascontiguousarray(bet_v.reshape(H2M, P).T).astype(np.float32)
    disP1_h = np.ascontiguousarray(dis[0].reshape(KT, P).T).astype(np.float32)
    rdisP_h = np.ascontiguousarray(rdis.reshape(KT, P).T).astype(np.float32)

    in_maps = []
    for r in range(NCORES):
        rows = slice(r * R, (r + 1) * R)
        adjp = []
        for A in A01:
            aT = A[rows, :].T.astype(NPF8)   # [NT, R] 0/1 fp8
            adjp.append(np.ascontiguousarray(
                aT.reshape(KT2, 2, P, R).transpose(0, 2, 1, 3)
                  .reshape(KT2 * P, 2 * R)))
        cu_h = [np.ascontiguousarray(
            np.broadcast_to(di[rows][None, :], (P, R))).astype(np.float32)
            for di in dis]
        disNM_h = np.ascontiguousarray(np.concatenate(
            [di[rows].reshape(RT, P).T for di in dis], axis=1)
        ).astype(np.float32)
        disRO_h = [np.ascontiguousarray(
            np.broadcast_to(di[rows][None, :], (O, R))).astype(np.float32)
            for di in dis]
        in_maps.append({
            "xTf": xTf_h,
            "xT": np.ascontiguousarray(x[rows].T).astype(np.float16),
            "adjp1": adjp[0], "adjp2": adjp[1],
            "wTe": wTe_h, "be": be_h, "bebc": bebc_h, "wTf": wTf_h,
            "bff": bff_h, "gam": gam_h, "bet": bet_h,
            "disP1": disP1_h, "rdisP": rdisP_h,
            "cu1": cu_h[0], "cu2": cu_h[1],
            "disNM": disNM_h,
            "disRO1": disRO_h[0], "disRO2": disRO_h[1],
        })
    return in_maps


def kernel(**inputs):
    NT, R = FULL_CFG["NT"], FULL_CFG["R"]
    nc = _get_program(NT, R)
    in_maps = make_in_maps(inputs, NT, R)
    res = run_bass_kernel_spmd(nc, in_maps, core_ids=list(range(NCORES)))
    out = np.concatenate(
        [res.results[r]["out"] for r in range(NCORES)], axis=0)
    return out.astype(np.float32)
